# revision 42
# baseline (speedup 1.0000x reference)
"""Multi-head attention (B=4, S=2048, D=1024, H=16) on 8 TRN2 NeuronCores.

Sharding: core c -> (batch b = c//2, head-group g = c%2 of 8 heads).
Data parallel over batch, tensor parallel over heads; each core computes
its group's QKV projection slices, causal attention for its 8 heads, and
the partial output projection. Host sums the two per-batch partials
(the tensor-parallel unshard) and adds the V-bias epilogue.

V2 layout/schedule notes:
- Emission is a one-stage software pipeline over 512-token sequence
  blocks: QK projection for block sb is emitted with that block's score
  matmuls + exps inlined (so the ACT engine is fed as soon as each
  head-pair's K chunk lands), the V projection follows, and the PV
  accumulation + normalization + output projection of block sb-1 are
  deferred until after block sb's scores -- they fill the PE's
  exp-bound gaps instead of delaying the next scores.
- Scores are computed in transposed form S.T[k, q] with two heads packed
  into the 128 partitions via row-tiled (tile_position) matmuls that run
  concurrently on the PE.
- The softmax denominator rides the PV matmul as a leading ones-column
  of V; normalization is a fast reciprocal + K=1 ones-broadcast matmul
  + DVE multiply per head.
- The K-projection bias is dropped entirely (a per-query additive
  constant on the scores is softmax-invariant); only the Q bias is
  applied, during the PSUM->SBUF move.
- Causal masking uses suffix-restricted score/exp/PV tiles plus a single
  shared [128,128] multiplicative 0/1 triangle strip applied on the
  exp'd probabilities; non-causal masks fall back to additive -1e9
  biases on the scores.
- Output projection results are cast f32->f16 and DMA'd per 128x512
  chunk; the host sums the two per-batch partials in f32.
"""

import os
import numpy as np

B, S, D, H = 4, 2048, 1024, 16
DK = D // H          # 64
HPC = H // 2         # heads per core = 8
GD = HPC * DK        # group feature width = 512
QT = 512             # q-tile width (free dim of S.T chunks)
KTL = 128            # k-tile length (partition dim of S.T chunks)
N_QT = S // QT       # 4
N_KT = S // KTL      # 16
SB = 512             # seq block (= QT)
NSB = S // SB        # 4
STRW = 128           # width of the multiplicative triangle strip
NEG = np.float32(-1e9)
SCALE = 1.0 / np.sqrt(np.float32(DK))

_cache = {}
last_results = None


def _classify_mask(mask2d):
    """Classify each (q-tile, k-tile) block of the [S,S] bool mask.

    Returns (plan, strips, biases):
      plan[qi] = list over valid kt of (kt, kind, a, b):
        kind 0 = clean (no masking)
        kind 1 = staircase: a = q0 (suffix start), b = (strip_idx, strip_w)
        kind 2 = general:   a = bias_idx
      strips: list of [KTL, STRW] f32 0/1 multiplicative masks
      biases: list of [KTL, QT] f32 additive -1e9/0 masks
    Blocks are in S.T (k, q) layout.
    """
    kl = np.arange(KTL)[:, None]
    ql = np.arange(QT)[None, :]
    plan = []
    strips, strip_keys = [], {}
    biases, bias_keys = [], {}
    for qi in range(N_QT):
        row = []
        for kt in range(N_KT):
            blk = mask2d[qi * QT:(qi + 1) * QT, kt * KTL:(kt + 1) * KTL].T
            if blk.all():
                continue
            if not blk.any():
                row.append((kt, 0, 0, None))
                continue
            dj = kt * KTL - qi * QT
            stair = (0 <= dj <= QT - KTL and np.array_equal(blk, kl + dj > ql)
                     and not os.environ.get("KERNEL_NO_STAIR"))
            q0 = dj if stair else 0
            if stair and (q0 == 0 or row):
                w = min(dj + KTL, QT) - q0    # = KTL for all causal stairs
                pat = (~blk[:, q0:q0 + w]).astype(np.float32)
                key = (w, pat.tobytes())
                if key not in strip_keys:
                    strip_keys[key] = len(strips)
                    p = np.zeros((KTL, STRW), np.float32)
                    p[:, :w] = pat
                    strips.append(p)
                row.append((kt, 1, q0, (strip_keys[key], w)))
            else:
                bias = np.where(blk, NEG, np.float32(0.0))
                key = bias.tobytes()
                if key not in bias_keys:
                    bias_keys[key] = len(biases)
                    biases.append(bias)
                row.append((kt, 2, bias_keys[key], None))
        if not row:
            # fully-masked q-row: include everything with full bias so the
            # softmax matches the reference's uniform distribution.
            bias = np.full((KTL, QT), NEG, np.float32)
            key = bias.tobytes()
            if key not in bias_keys:
                bias_keys[key] = len(biases)
                biases.append(bias)
            row = [(kt, 2, bias_keys[key], None) for kt in range(N_KT)]
        plan.append(row)
    return plan, strips, biases


def _build(plan, n_strips, n_biases):
    import concourse.bass as bass
    import concourse.bacc as bacc
    import concourse.tile as tile
    import concourse.mybir as mybir
    from contextlib import ExitStack

    f32 = mybir.dt.float32
    f32r = mybir.dt.float32r
    f16 = mybir.dt.float16
    Exp = mybir.ActivationFunctionType.Exp

    nc = bacc.Bacc(trn_type="TRN2", target_bir_lowering=False, debug=False)
    xT = nc.dram_tensor("xT", [D, S], f16, kind="ExternalInput").ap()
    w_qk = nc.dram_tensor("w_qk", [D, 2 * GD], f16, kind="ExternalInput").ap()
    b_q = nc.dram_tensor("b_q", [GD], f32, kind="ExternalInput").ap()
    w_v = nc.dram_tensor("w_v", [D, GD], f16, kind="ExternalInput").ap()
    wo_T = nc.dram_tensor("wo_T", [GD, D], f16, kind="ExternalInput").ap()
    maskm = nc.dram_tensor("maskm", [max(n_strips, 1), KTL, STRW], f16,
                           kind="ExternalInput").ap()
    maskb = nc.dram_tensor("maskb", [max(n_biases, 1), KTL, QT], f32,
                           kind="ExternalInput").ap()
    outT = nc.dram_tensor("outT", [D, S], f16, kind="ExternalOutput").ap()
    ND = D // 128        # 8 contraction chunks
    NK3 = GD // 128      # 4 output-projection contraction chunks
    debug = bool(os.environ.get("KERNEL_DEBUG"))
    if debug:
        dbg_ot = nc.dram_tensor("dbg_ot", [128, 2, QT], f32, kind="ExternalOutput").ap()
        dbg_rb = nc.dram_tensor("dbg_rb", [128, QT], f16, kind="ExternalOutput").ap()
        dbg_r2 = nc.dram_tensor("dbg_r2", [64, QT], f32, kind="ExternalOutput").ap()
        dbg_pt = nc.dram_tensor("dbg_pt", [4, 128, 2, QT], f16, kind="ExternalOutput").ap()
        dbg_otq = nc.dram_tensor("dbg_otq", [128, QT], f16, kind="ExternalOutput").ap()
        dbg_q = nc.dram_tensor("dbg_q", [NK3, 128, S], f16, kind="ExternalOutput").ap()
        dbg_x = nc.dram_tensor("dbg_x", [ND, 128, S], f16, kind="ExternalOutput").ap()
        dbg_k = nc.dram_tensor("dbg_k", [NK3, 128, S], f16, kind="ExternalOutput").ap()
        dbg_v = nc.dram_tensor("dbg_v", [N_KT, 128, HPC, 128], f16, kind="ExternalOutput").ap()
    M_ORDER = [0, 4, 1, 5, 2, 6, 3, 7]   # Q/K alternating for early scores

    with tile.TileContext(nc) as tc, ExitStack() as ctx:
        singles = ctx.enter_context(tc.tile_pool(name="singles", bufs=1))
        qkt_pool = ctx.enter_context(tc.tile_pool(name="qkt", bufs=1))
        v_pool = ctx.enter_context(tc.tile_pool(name="vp", bufs=1))
        x_pool = ctx.enter_context(tc.tile_pool(name="xp", bufs=1))
        w_pool = ctx.enter_context(tc.tile_pool(name="wp", bufs=1))
        fill_ps = ctx.enter_context(tc.tile_pool(name="fill", bufs=2,
                                                 space="PSUM"))
        st_ps = ctx.enter_context(tc.tile_pool(name="st", bufs=2, space="PSUM"))
        ot_ps = ctx.enter_context(tc.tile_pool(name="ot", bufs=1, space="PSUM"))
        pt_pool = ctx.enter_context(tc.tile_pool(name="pt", bufs=23))
        otq_pool = ctx.enter_context(tc.tile_pool(name="otq", bufs=2))
        ob_pool = ctx.enter_context(tc.tile_pool(name="ob", bufs=3))
        r2_pool = ctx.enter_context(tc.tile_pool(name="r2", bufs=4))

        # ---- static tiles + loads (weights first: they gate the start) ----
        bq_t = singles.tile([128, NK3], f32)
        nc.sync.dma_start(out=bq_t, in_=b_q.rearrange("(m p) -> p m", p=128))
        ones1 = singles.tile([1, 64], f16)
        nc.vector.memset(ones1, 1.0)
        ones128 = singles.tile([1, 128], f16)
        nc.vector.memset(ones128, 1.0)
        warm = singles.tile([1, QT], f16)
        nc.vector.memset(warm, 1.0)
        for _ in range(16):
            wps = fill_ps.tile([128, QT], f32, tag="fill", name="warm_ps")
            nc.tensor.matmul(wps[:], ones128[:], warm[:], start=True,
                             stop=True)

        mm_t = []
        for i in range(n_strips):
            t = singles.tile([KTL, STRW], f16, tag=f"mm{i}", name=f"mm{i}")
            nc.sync.dma_start(out=t, in_=maskm[i])
            mm_t.append(t)
        mb_t = []
        for i in range(n_biases):
            t = singles.tile([KTL, QT], f32, tag=f"mb{i}", name=f"mb{i}")
            nc.sync.dma_start(out=t, in_=maskb[i])
            mb_t.append(t)

        # x chunks, per (k, sb) for fine-grained deps
        xs = [[x_pool.tile([128, SB], f16, tag=f"x{k}_{sb}",
                           name=f"x{k}_{sb}") for sb in range(NSB)]
              for k in range(ND)]
        wqk_t = [[w_pool.tile([128, GD], f16, tag=f"wqk{k}_{h}",
                              name=f"wqk{k}_{h}") for h in range(2)]
                 for k in range(ND)]
        wv_t = [w_pool.tile([128, GD], f16, tag=f"wv{k}", name=f"wv{k}")
                for k in range(ND)]
        wo_t = [w_pool.tile([128, D], f16, tag=f"wo{k}", name=f"wo{k}")
                for k in range(NK3)]
        # load order = what gates the start first: Q weights + first x
        # block, then K weights, V weights, the rest of x, then wo.
        for k in range(ND):
            nc.sync.dma_start(out=wqk_t[k][0],
                              in_=w_qk[128 * k:128 * (k + 1), 0:GD])
            nc.sync.dma_start(out=xs[k][0],
                              in_=xT[128 * k:128 * (k + 1), 0:SB])
            nc.sync.dma_start(out=wqk_t[k][1],
                              in_=w_qk[128 * k:128 * (k + 1), GD:2 * GD])
        for k in range(ND):
            nc.sync.dma_start(out=wv_t[k], in_=w_v[128 * k:128 * (k + 1)])
        for sb in range(1, NSB):
            for k in range(ND):
                nc.sync.dma_start(
                    out=xs[k][sb],
                    in_=xT[128 * k:128 * (k + 1), SB * sb:SB * (sb + 1)])
        for k in range(NK3):
            nc.sync.dma_start(out=wo_t[k], in_=wo_T[128 * k:128 * (k + 1)])

        # Q.T / K.T, per (hp, sb); rows 0:64 = head 2hp, 64:128 = head 2hp+1
        qkt_q = [[qkt_pool.tile([128, SB], f16, tag=f"qq{hp}_{sb}",
                                name=f"qq{hp}_{sb}") for sb in range(NSB)]
                 for hp in range(NK3)]
        qkt_k = [[qkt_pool.tile([128, SB], f16, tag=f"qk{hp}_{sb}",
                                name=f"qk{hp}_{sb}") for sb in range(NSB)]
                 for hp in range(NK3)]
        # V tiles: [128 tok, 8 heads, 128] = [ones | zeros(63) | V(64)]
        # per head: the leading ones column makes the PV matmul emit the
        # softmax denominator on partition 0; V outputs land on 64:128.
        v_sb = [v_pool.tile([128, HPC, 128], f16, tag=f"v{t}", name=f"v{t}")
                for t in range(N_KT)]
        for t in range(N_KT):
            nc.vector.memset(v_sb[t][:, :, 0:1], 1.0)
            nc.vector.memset(v_sb[t][:, :, 1:64], 0.0)

        def emit_scores_exp(qi, hp):
            """scores -> exp (+mask) for one (q-block, head-pair).
            Returns the pt tiles for the deferred PV pass."""
            kts = plan[qi]
            pts = []
            for ki, (kt, kind, a, bopt) in enumerate(kts):
                q0 = a if kind == 1 else 0
                st = st_ps.tile([128, 2, QT], f32, tag="st", name="st")
                for h in range(2):
                    lo = 64 * h
                    nc.tensor.matmul(
                        st[:, h, q0:QT],
                        qkt_k[hp][kt // 4][lo:lo + 64,
                                           KTL * (kt % 4):KTL * (kt % 4 + 1)],
                        qkt_q[hp][qi][lo:lo + 64, q0:QT],
                        start=True, stop=True, tile_position=(lo, 0))
                if kind == 2:
                    for h in range(2):
                        nc.vector.tensor_add(st[:, h, :], st[:, h, :], mb_t[a])
                pt = pt_pool.tile([128, 2, QT], f16, tag="pt", name="pt")
                nc.scalar.activation(out=pt[:, :, q0:QT], in_=st[:, :, q0:QT],
                                     func=Exp, scale=float(SCALE))
                if kind == 1:
                    si, w = bopt
                    for h in range(2):
                        nc.gpsimd.tensor_mul(pt[:, h, q0:q0 + w],
                                             pt[:, h, q0:q0 + w],
                                             mm_t[si][:, 0:w])
                pts.append(pt)
            return pts

        def emit_pv_norm(qi, hp, pts, otq):
            """PV accumulation + normalization; pts from emit_scores_exp."""
            kts = plan[qi]
            ot = [ot_ps.tile([128, QT], f32, tag=f"ot{h}", name=f"ot{h}")
                  for h in range(2)]
            for ki, (kt, kind, a, bopt) in enumerate(kts):
                q0 = a if kind == 1 else 0
                for h in range(2):
                    nc.tensor.matmul(
                        ot[h][:, q0:QT],
                        v_sb[kt][:, 2 * hp + h, :],
                        pts[ki][:, h, q0:QT],
                        start=(ki == 0), stop=(ki == len(kts) - 1))
            if debug and qi == 3 and hp == 0:
                for di, ki in enumerate((0, 7, 11, 15)):
                    nc.sync.dma_start(out=dbg_pt[di], in_=pts[ki][:])
                ot_dbg = ob_pool.tile([128, 2, QT], f32, tag="otdbg", name="ot_dbg")
                nc.vector.tensor_copy(out=ot_dbg[0:65, 0, :], in_=ot[0][0:65, :])
                nc.vector.tensor_copy(out=ot_dbg[0:65, 1, :], in_=ot[1][0:65, :])
                nc.sync.dma_start(out=dbg_ot, in_=ot_dbg)
            # ---- normalization (baseline scheme, per h) ----
            for h in range(2):
                r_row = r2_pool.tile([1, QT], f32, tag="rrow", name="r_row")
                nc.vector.reciprocal_approx_fast(out=r_row[:],
                                                 in_=ot[h][0:1, :])
                r16 = r2_pool.tile([1, QT], f16, tag="r16", name="r16")
                nc.vector.tensor_copy(out=r16[:], in_=r_row[:])
                rb_ps = fill_ps.tile([128, QT], f32, tag="fill", name="rb_ps")
                nc.tensor.matmul(rb_ps[0:64, :], ones1[:], r16[:],
                                 start=True, stop=True)
                rb_sb = r2_pool.tile([64, QT], f32, tag="rbsb", name="rb_sb")
                nc.vector.tensor_copy(out=rb_sb, in_=rb_ps[0:64, :])
                nc.vector.tensor_mul(otq[hp][64 * h:64 * h + 64, :],
                                     ot[h][64:128, :], rb_sb[:])
            if debug and qi == 3 and hp == 0:
                nc.sync.dma_start(out=dbg_otq, in_=otq[hp][:])

        def emit_qkv(sb, pts_out):
            # Q/K first, alternating, with scores+exp emitted inline as
            # soon as each head-pair's K chunk lands (keeps ACT fed); the
            # PV pass is deferred until after the V projection below.
            for m in M_ORDER:
                half, mc = divmod(m, 4)
                pss = fill_ps.tile([128, SB], f32, tag="fill", name="pss")
                for k in range(ND):
                    nc.tensor.matmul(
                        pss[:], wqk_t[k][half][:, 128 * mc:128 * (mc + 1)],
                        xs[k][sb][:], start=(k == 0), stop=(k == ND - 1))
                if half == 0:
                    nc.vector.tensor_scalar_add(qkt_q[mc][sb][:], pss[:],
                                                bq_t[:, mc:mc + 1])
                else:
                    nc.vector.tensor_copy(out=qkt_k[mc][sb][:], in_=pss[:])
                    if pts_out is not None:
                        pts_out[mc] = emit_scores_exp(sb, mc)
            for tt in range(SB // 128):
                t = sb * (SB // 128) + tt
                ps = fill_ps.tile([128, GD], f32, tag="fill", name="ps_v")
                for k in range(ND):
                    nc.tensor.matmul(
                        ps[:], xs[k][sb][:, 128 * tt:128 * (tt + 1)],
                        wv_t[k][:], start=(k == 0), stop=(k == ND - 1))
                nc.vector.tensor_copy(
                    out=v_sb[t][:, :, 64:128],
                    in_=ps[:].rearrange("p (h d) -> p h d", h=HPC))

        def emit_op(qi, otq):
            for m in range(D // 128):
                ps = fill_ps.tile([128, QT], f32, tag="fill", name="ps_o")
                for k in range(NK3):
                    nc.tensor.matmul(
                        ps[:], wo_t[k][:, 128 * m:128 * (m + 1)], otq[k][:],
                        start=(k == 0), stop=(k == NK3 - 1))
                ob = ob_pool.tile([128, QT], f16, tag="ob", name="ob")
                nc.vector.tensor_copy(out=ob[:], in_=ps[:])
                nc.sync.dma_start(
                    out=outT[128 * m:128 * (m + 1), QT * qi:QT * (qi + 1)],
                    in_=ob[:])

        if os.environ.get("KERNEL_SEQ"):
            # baseline-like phase order (debug): all QKV, then attention+OP
            for sb in range(NSB):
                emit_qkv(sb, None)
            for qi in range(N_QT):
                otq = [otq_pool.tile([128, QT], f16, tag=f"otq{c}",
                                     name=f"otq{c}") for c in range(NK3)]
                for hp in range(NK3):
                    emit_pv_norm(qi, hp, emit_scores_exp(qi, hp), otq)
                emit_op(qi, otq)
        else:
            # one-stage software pipeline: PV/norm/OP for block sb are
            # emitted after block sb+1's QK+scores, so the next scores
            # (which feed the ACT engine) outrank them in the ready heap.
            pending = None
            for sb in range(NSB):
                otq = [otq_pool.tile([128, QT], f16, tag=f"otq{c}",
                                     name=f"otq{c}") for c in range(NK3)]
                pts_sb = [None] * NK3
                emit_qkv(sb, pts_sb)
                if pending is not None:
                    p_sb, p_pts, p_otq = pending
                    for hp in range(NK3):
                        emit_pv_norm(p_sb, hp, p_pts[hp], p_otq)
                    emit_op(p_sb, p_otq)
                pending = (sb, pts_sb, otq)
            p_sb, p_pts, p_otq = pending
            for hp in range(NK3):
                emit_pv_norm(p_sb, hp, p_pts[hp], p_otq)
            emit_op(p_sb, p_otq)
        if debug:
            for hp in range(NK3):
                for sb in range(NSB):
                    nc.sync.dma_start(out=dbg_q[hp, :, SB * sb:SB * (sb + 1)],
                                      in_=qkt_q[hp][sb][:])
                    nc.sync.dma_start(out=dbg_k[hp, :, SB * sb:SB * (sb + 1)],
                                      in_=qkt_k[hp][sb][:])
            for k in range(ND):
                nc.sync.dma_start(out=dbg_x[k], in_=xs[k][:])
            for t in range(N_KT):
                nc.sync.dma_start(out=dbg_v[t], in_=v_sb[t][:])
    nc.compile()
    return nc


def kernel(encodings_for_qkv, mask, w_qkv, b_qkv, w_o):
    global last_results
    from concourse.bass_utils import run_bass_kernel_spmd

    x = np.ascontiguousarray(np.asarray(encodings_for_qkv, dtype=np.float32))
    mask2d = np.asarray(mask).reshape(S, S).astype(bool)
    w_qkv = np.asarray(w_qkv, dtype=np.float32)
    b_qkv = np.asarray(b_qkv, dtype=np.float32)
    w_o = np.asarray(w_o, dtype=np.float32)

    plan, strips, biases = _classify_mask(mask2d)
    key = repr([[e[:3] + ((e[3][0], e[3][1]) if e[3] else None,) for e in row]
                for row in plan]) + repr(sorted(
                    (k, v) for k, v in os.environ.items() if k.startswith("KERNEL_")))
    if key not in _cache:
        _cache[key] = _build(plan, len(strips), len(biases))
    nc = _cache[key]

    maskm = (np.stack(strips) if strips
             else np.zeros((1, KTL, STRW), dtype=np.float32))
    maskb = (np.stack(biases) if biases
             else np.zeros((1, KTL, QT), dtype=np.float32))
    wT = np.ascontiguousarray(w_qkv.T)        # [D, 3D]
    woT_full = w_o.T                          # [D(in), D(out)]

    in_maps = []
    for c in range(8):
        b, g = divmod(c, 2)
        cols = slice(GD * g, GD * (g + 1))
        w_qk_g = np.ascontiguousarray(
            np.concatenate([wT[:, 0 * D:][:, cols], wT[:, 1 * D:][:, cols]], axis=1))
        b_q_g = np.ascontiguousarray(b_qkv[0 * D:1 * D][cols])
        w_v_g = np.ascontiguousarray(wT[:, 2 * D:][:, cols])
        wo_T_g = np.ascontiguousarray(woT_full[cols, :])
        in_maps.append({
            "xT": np.ascontiguousarray(x[b].T).astype(np.float16),
            "w_qk": w_qk_g.astype(np.float16), "b_q": b_q_g,
            "w_v": w_v_g.astype(np.float16),
            "wo_T": wo_T_g.astype(np.float16),
            "maskm": maskm.astype(np.float16), "maskb": maskb,
        })

    trace = bool(int(os.environ.get("KERNEL_PROFILE", "0")))
    res = run_bass_kernel_spmd(nc, in_maps, core_ids=list(range(8)),
                               trace=trace,
                               trace_cores=list(range(8)) if trace else None)
    last_results = res

    out = np.empty((B, S, D), dtype=np.float32)
    for b in range(B):
        acc = (res.results[2 * b]["outT"].astype(np.float32)
               + res.results[2 * b + 1]["outT"].astype(np.float32))
        out[b] = acc.T
    # V-bias epilogue: softmax rows sum to 1, so the V bias contributes a
    # constant (b_v @ w_o.T) to every sequence position.
    out += (b_qkv[2 * D:] @ woT_full).reshape(1, 1, D)
    return out


# revision 43
# speedup vs baseline: 1.0187x; 1.0187x over previous
"""Multi-head attention (B=4, S=2048, D=1024, H=16) on 8 TRN2 NeuronCores.

Sharding: core c -> (batch b = c//2, head-group g = c%2 of 8 heads).
Data parallel over batch, tensor parallel over heads; each core computes
its group's QKV projection slices, causal attention for its 8 heads, and
the partial output projection. Host sums the two per-batch partials
(the tensor-parallel unshard) and adds the V-bias epilogue.

V2 layout/schedule notes:
- Emission is a one-stage software pipeline over 512-token sequence
  blocks: QK projection for block sb is emitted with that block's score
  matmuls + exps inlined (so the ACT engine is fed as soon as each
  head-pair's K chunk lands), the V projection follows, and the PV
  accumulation + normalization + output projection of block sb-1 are
  deferred until after block sb's scores -- they fill the PE's
  exp-bound gaps instead of delaying the next scores.
- Scores are computed in transposed form S.T[k, q] with two heads packed
  into the 128 partitions via row-tiled (tile_position) matmuls that run
  concurrently on the PE.
- The softmax denominator rides the PV matmul as a leading ones-column
  of V; normalization is a fast reciprocal + K=1 ones-broadcast matmul
  + DVE multiply per head.
- The K-projection bias is dropped entirely (a per-query additive
  constant on the scores is softmax-invariant); only the Q bias is
  applied, during the PSUM->SBUF move.
- Causal masking uses suffix-restricted score/exp/PV tiles plus a single
  shared [128,128] multiplicative 0/1 triangle strip applied on the
  exp'd probabilities; non-causal masks fall back to additive -1e9
  biases on the scores.
- Output projection results are cast f32->f16 and DMA'd per 128x512
  chunk; the host sums the two per-batch partials in f32.
"""

import os
import numpy as np

B, S, D, H = 4, 2048, 1024, 16
DK = D // H          # 64
HPC = H // 2         # heads per core = 8
GD = HPC * DK        # group feature width = 512
QT = 512             # q-tile width (free dim of S.T chunks)
KTL = 128            # k-tile length (partition dim of S.T chunks)
N_QT = S // QT       # 4
N_KT = S // KTL      # 16
SB = 512             # seq block (= QT)
NSB = S // SB        # 4
STRW = 128           # width of the multiplicative triangle strip
NEG = np.float32(-1e9)
SCALE = 1.0 / np.sqrt(np.float32(DK))

_cache = {}
last_results = None


def _classify_mask(mask2d):
    """Classify each (q-tile, k-tile) block of the [S,S] bool mask.

    Returns (plan, strips, biases):
      plan[qi] = list over valid kt of (kt, kind, a, b):
        kind 0 = clean (no masking)
        kind 1 = staircase: a = q0 (suffix start), b = (strip_idx, strip_w)
        kind 2 = general:   a = bias_idx
      strips: list of [KTL, STRW] f32 0/1 multiplicative masks
      biases: list of [KTL, QT] f32 additive -1e9/0 masks
    Blocks are in S.T (k, q) layout.
    """
    kl = np.arange(KTL)[:, None]
    ql = np.arange(QT)[None, :]
    plan = []
    strips, strip_keys = [], {}
    biases, bias_keys = [], {}
    for qi in range(N_QT):
        row = []
        for kt in range(N_KT):
            blk = mask2d[qi * QT:(qi + 1) * QT, kt * KTL:(kt + 1) * KTL].T
            if blk.all():
                continue
            if not blk.any():
                row.append((kt, 0, 0, None))
                continue
            dj = kt * KTL - qi * QT
            stair = (0 <= dj <= QT - KTL and np.array_equal(blk, kl + dj > ql)
                     and not os.environ.get("KERNEL_NO_STAIR"))
            q0 = dj if stair else 0
            if stair and (q0 == 0 or row):
                w = min(dj + KTL, QT) - q0    # = KTL for all causal stairs
                pat = (~blk[:, q0:q0 + w]).astype(np.float32)
                key = (w, pat.tobytes())
                if key not in strip_keys:
                    strip_keys[key] = len(strips)
                    p = np.zeros((KTL, STRW), np.float32)
                    p[:, :w] = pat
                    strips.append(p)
                row.append((kt, 1, q0, (strip_keys[key], w)))
            else:
                bias = np.where(blk, NEG, np.float32(0.0))
                key = bias.tobytes()
                if key not in bias_keys:
                    bias_keys[key] = len(biases)
                    biases.append(bias)
                row.append((kt, 2, bias_keys[key], None))
        if not row:
            # fully-masked q-row: include everything with full bias so the
            # softmax matches the reference's uniform distribution.
            bias = np.full((KTL, QT), NEG, np.float32)
            key = bias.tobytes()
            if key not in bias_keys:
                bias_keys[key] = len(biases)
                biases.append(bias)
            row = [(kt, 2, bias_keys[key], None) for kt in range(N_KT)]
        plan.append(row)
    return plan, strips, biases


def _build(plan, n_strips, n_biases):
    import concourse.bass as bass
    import concourse.bacc as bacc
    import concourse.tile as tile
    import concourse.mybir as mybir
    from contextlib import ExitStack

    f32 = mybir.dt.float32
    f32r = mybir.dt.float32r
    f16 = mybir.dt.float16
    Exp = mybir.ActivationFunctionType.Exp

    nc = bacc.Bacc(trn_type="TRN2", target_bir_lowering=False, debug=False)
    xT = nc.dram_tensor("xT", [D, S], f16, kind="ExternalInput").ap()
    w_qk = nc.dram_tensor("w_qk", [D, 2 * GD], f16, kind="ExternalInput").ap()
    b_q = nc.dram_tensor("b_q", [GD], f32, kind="ExternalInput").ap()
    w_v = nc.dram_tensor("w_v", [D, GD], f16, kind="ExternalInput").ap()
    wo_T = nc.dram_tensor("wo_T", [GD, D], f16, kind="ExternalInput").ap()
    maskm = nc.dram_tensor("maskm", [max(n_strips, 1), KTL, STRW], f16,
                           kind="ExternalInput").ap()
    maskb = nc.dram_tensor("maskb", [max(n_biases, 1), KTL, QT], f32,
                           kind="ExternalInput").ap()
    outT = nc.dram_tensor("outT", [D, S], f16, kind="ExternalOutput").ap()
    ND = D // 128        # 8 contraction chunks
    NK3 = GD // 128      # 4 output-projection contraction chunks
    debug = bool(os.environ.get("KERNEL_DEBUG"))
    if debug:
        dbg_ot = nc.dram_tensor("dbg_ot", [128, 2, QT], f32, kind="ExternalOutput").ap()
        dbg_rb = nc.dram_tensor("dbg_rb", [128, QT], f16, kind="ExternalOutput").ap()
        dbg_r2 = nc.dram_tensor("dbg_r2", [64, QT], f32, kind="ExternalOutput").ap()
        dbg_pt = nc.dram_tensor("dbg_pt", [4, 128, 2, QT], f16, kind="ExternalOutput").ap()
        dbg_otq = nc.dram_tensor("dbg_otq", [128, QT], f16, kind="ExternalOutput").ap()
        dbg_q = nc.dram_tensor("dbg_q", [NK3, 128, S], f16, kind="ExternalOutput").ap()
        dbg_x = nc.dram_tensor("dbg_x", [ND, 128, S], f16, kind="ExternalOutput").ap()
        dbg_k = nc.dram_tensor("dbg_k", [NK3, 128, S], f16, kind="ExternalOutput").ap()
        dbg_v = nc.dram_tensor("dbg_v", [N_KT, 128, HPC, 128], f16, kind="ExternalOutput").ap()
    M_ORDER = [0, 4, 1, 5, 2, 6, 3, 7]   # Q/K alternating for early scores

    with tile.TileContext(nc) as tc, ExitStack() as ctx:
        singles = ctx.enter_context(tc.tile_pool(name="singles", bufs=1))
        qkt_pool = ctx.enter_context(tc.tile_pool(name="qkt", bufs=1))
        v_pool = ctx.enter_context(tc.tile_pool(name="vp", bufs=1))
        x_pool = ctx.enter_context(tc.tile_pool(name="xp", bufs=1))
        w_pool = ctx.enter_context(tc.tile_pool(name="wp", bufs=1))
        fill_ps = ctx.enter_context(tc.tile_pool(name="fill", bufs=2,
                                                 space="PSUM"))
        st_ps = ctx.enter_context(tc.tile_pool(name="st", bufs=2, space="PSUM"))
        ot_ps = ctx.enter_context(tc.tile_pool(name="ot", bufs=1, space="PSUM"))
        pt_pool = ctx.enter_context(tc.tile_pool(name="pt", bufs=23))
        otq_pool = ctx.enter_context(tc.tile_pool(name="otq", bufs=2))
        ob_pool = ctx.enter_context(tc.tile_pool(name="ob", bufs=3))
        r2_pool = ctx.enter_context(tc.tile_pool(name="r2", bufs=4))

        # ---- static tiles + loads (weights first: they gate the start) ----
        bq_t = singles.tile([128, NK3], f32)
        nc.sync.dma_start(out=bq_t, in_=b_q.rearrange("(m p) -> p m", p=128))
        ones1 = singles.tile([1, 64], f16)
        nc.vector.memset(ones1, 1.0)
        ones128 = singles.tile([1, 128], f16)
        nc.vector.memset(ones128, 1.0)
        warm = singles.tile([1, QT], f16)
        nc.vector.memset(warm, 1.0)
        for _ in range(16):
            wps = fill_ps.tile([128, QT], f32, tag="fill", name="warm_ps")
            nc.tensor.matmul(wps[:], ones128[:], warm[:], start=True,
                             stop=True)

        mm_t = []
        for i in range(n_strips):
            t = singles.tile([KTL, STRW], f16, tag=f"mm{i}", name=f"mm{i}")
            nc.sync.dma_start(out=t, in_=maskm[i])
            mm_t.append(t)
        mb_t = []
        for i in range(n_biases):
            t = singles.tile([KTL, QT], f32, tag=f"mb{i}", name=f"mb{i}")
            nc.sync.dma_start(out=t, in_=maskb[i])
            mb_t.append(t)

        # x chunks, per (k, sb) for fine-grained deps
        xs = [[x_pool.tile([128, SB], f16, tag=f"x{k}_{sb}",
                           name=f"x{k}_{sb}") for sb in range(NSB)]
              for k in range(ND)]
        wqk_t = [[w_pool.tile([128, GD], f16, tag=f"wqk{k}_{h}",
                              name=f"wqk{k}_{h}") for h in range(2)]
                 for k in range(ND)]
        wv_t = [w_pool.tile([128, GD], f16, tag=f"wv{k}", name=f"wv{k}")
                for k in range(ND)]
        wo_t = [w_pool.tile([128, D], f16, tag=f"wo{k}", name=f"wo{k}")
                for k in range(NK3)]
        # load order = what gates the start first: Q weights + first x
        # block, then K weights, V weights, the rest of x, then wo.
        for k in range(ND):
            nc.sync.dma_start(out=wqk_t[k][0],
                              in_=w_qk[128 * k:128 * (k + 1), 0:GD])
            nc.sync.dma_start(out=xs[k][0],
                              in_=xT[128 * k:128 * (k + 1), 0:SB])
            nc.sync.dma_start(out=wqk_t[k][1],
                              in_=w_qk[128 * k:128 * (k + 1), GD:2 * GD])
        for k in range(ND):
            nc.sync.dma_start(out=wv_t[k], in_=w_v[128 * k:128 * (k + 1)])
        for sb in range(1, NSB):
            for k in range(ND):
                nc.sync.dma_start(
                    out=xs[k][sb],
                    in_=xT[128 * k:128 * (k + 1), SB * sb:SB * (sb + 1)])
        for k in range(NK3):
            nc.sync.dma_start(out=wo_t[k], in_=wo_T[128 * k:128 * (k + 1)])

        # Q.T / K.T, per (hp, sb); rows 0:64 = head 2hp, 64:128 = head 2hp+1
        qkt_q = [[qkt_pool.tile([128, SB], f16, tag=f"qq{hp}_{sb}",
                                name=f"qq{hp}_{sb}") for sb in range(NSB)]
                 for hp in range(NK3)]
        qkt_k = [[qkt_pool.tile([128, SB], f16, tag=f"qk{hp}_{sb}",
                                name=f"qk{hp}_{sb}") for sb in range(NSB)]
                 for hp in range(NK3)]
        # V tiles: [128 tok, 8 heads, 128] = [ones | zeros(63) | V(64)]
        # per head: the leading ones column makes the PV matmul emit the
        # softmax denominator on partition 0; V outputs land on 64:128.
        v_sb = [v_pool.tile([128, HPC, 128], f16, tag=f"v{t}", name=f"v{t}")
                for t in range(N_KT)]
        for t in range(N_KT):
            nc.vector.memset(v_sb[t][:, :, 0:1], 1.0)
            nc.vector.memset(v_sb[t][:, :, 1:64], 0.0)

        def emit_scores_exp(qi, hp):
            """scores -> exp (+mask) for one (q-block, head-pair).
            Returns the pt tiles for the deferred PV pass."""
            kts = plan[qi]
            pts = []
            for ki, (kt, kind, a, bopt) in enumerate(kts):
                q0 = a if kind == 1 else 0
                st = st_ps.tile([128, 2, QT], f32, tag="st", name="st")
                for h in range(2):
                    lo = 64 * h
                    nc.tensor.matmul(
                        st[:, h, q0:QT],
                        qkt_k[hp][kt // 4][lo:lo + 64,
                                           KTL * (kt % 4):KTL * (kt % 4 + 1)],
                        qkt_q[hp][qi][lo:lo + 64, q0:QT],
                        start=True, stop=True, tile_position=(lo, 0))
                if kind == 2:
                    for h in range(2):
                        nc.vector.tensor_add(st[:, h, :], st[:, h, :], mb_t[a])
                pt = pt_pool.tile([128, 2, QT], f16, tag="pt", name="pt")
                nc.scalar.activation(out=pt[:, :, q0:QT], in_=st[:, :, q0:QT],
                                     func=Exp, scale=float(SCALE))
                if kind == 1:
                    si, w = bopt
                    for h in range(2):
                        nc.vector.tensor_mul(pt[:, h, q0:q0 + w],
                                             pt[:, h, q0:q0 + w],
                                             mm_t[si][:, 0:w])
                pts.append(pt)
            return pts

        def emit_pv_norm(qi, hp, pts, otq):
            """PV accumulation + normalization; pts from emit_scores_exp."""
            kts = plan[qi]
            ot = [ot_ps.tile([128, QT], f32, tag=f"ot{h}", name=f"ot{h}")
                  for h in range(2)]
            for ki, (kt, kind, a, bopt) in enumerate(kts):
                q0 = a if kind == 1 else 0
                for h in range(2):
                    nc.tensor.matmul(
                        ot[h][:, q0:QT],
                        v_sb[kt][:, 2 * hp + h, :],
                        pts[ki][:, h, q0:QT],
                        start=(ki == 0), stop=(ki == len(kts) - 1))
            if debug and qi == 3 and hp == 0:
                for di, ki in enumerate((0, 7, 11, 15)):
                    nc.sync.dma_start(out=dbg_pt[di], in_=pts[ki][:])
                ot_dbg = ob_pool.tile([128, 2, QT], f32, tag="otdbg", name="ot_dbg")
                nc.vector.tensor_copy(out=ot_dbg[0:65, 0, :], in_=ot[0][0:65, :])
                nc.vector.tensor_copy(out=ot_dbg[0:65, 1, :], in_=ot[1][0:65, :])
                nc.sync.dma_start(out=dbg_ot, in_=ot_dbg)
            # ---- normalization (baseline scheme, per h) ----
            for h in range(2):
                r_row = r2_pool.tile([1, QT], f32, tag="rrow", name="r_row")
                nc.vector.reciprocal_approx_fast(out=r_row[:],
                                                 in_=ot[h][0:1, :])
                r16 = r2_pool.tile([1, QT], f16, tag="r16", name="r16")
                nc.vector.tensor_copy(out=r16[:], in_=r_row[:])
                rb_ps = fill_ps.tile([128, QT], f32, tag="fill", name="rb_ps")
                nc.tensor.matmul(rb_ps[0:64, :], ones1[:], r16[:],
                                 start=True, stop=True)
                rb_sb = r2_pool.tile([64, QT], f32, tag="rbsb", name="rb_sb")
                nc.vector.tensor_copy(out=rb_sb, in_=rb_ps[0:64, :])
                nc.vector.tensor_mul(otq[hp][64 * h:64 * h + 64, :],
                                     ot[h][64:128, :], rb_sb[:])
            if debug and qi == 3 and hp == 0:
                nc.sync.dma_start(out=dbg_otq, in_=otq[hp][:])

        def emit_qkv(sb, pts_out):
            # Q/K first, alternating, with scores+exp emitted inline as
            # soon as each head-pair's K chunk lands (keeps ACT fed); the
            # PV pass is deferred until after the V projection below.
            for m in M_ORDER:
                half, mc = divmod(m, 4)
                pss = fill_ps.tile([128, SB], f32, tag="fill", name="pss")
                for k in range(ND):
                    nc.tensor.matmul(
                        pss[:], wqk_t[k][half][:, 128 * mc:128 * (mc + 1)],
                        xs[k][sb][:], start=(k == 0), stop=(k == ND - 1))
                if half == 0:
                    nc.vector.tensor_scalar_add(qkt_q[mc][sb][:], pss[:],
                                                bq_t[:, mc:mc + 1])
                else:
                    nc.vector.tensor_copy(out=qkt_k[mc][sb][:], in_=pss[:])
                    if pts_out is not None:
                        pts_out[mc] = emit_scores_exp(sb, mc)
            for tt in range(SB // 128):
                t = sb * (SB // 128) + tt
                ps = fill_ps.tile([128, GD], f32, tag="fill", name="ps_v")
                for k in range(ND):
                    nc.tensor.matmul(
                        ps[:], xs[k][sb][:, 128 * tt:128 * (tt + 1)],
                        wv_t[k][:], start=(k == 0), stop=(k == ND - 1))
                nc.vector.tensor_copy(
                    out=v_sb[t][:, :, 64:128],
                    in_=ps[:].rearrange("p (h d) -> p h d", h=HPC))

        def emit_op(qi, otq):
            for m in range(D // 128):
                ps = fill_ps.tile([128, QT], f32, tag="fill", name="ps_o")
                for k in range(NK3):
                    nc.tensor.matmul(
                        ps[:], wo_t[k][:, 128 * m:128 * (m + 1)], otq[k][:],
                        start=(k == 0), stop=(k == NK3 - 1))
                ob = ob_pool.tile([128, QT], f16, tag="ob", name="ob")
                nc.vector.tensor_copy(out=ob[:], in_=ps[:])
                nc.sync.dma_start(
                    out=outT[128 * m:128 * (m + 1), QT * qi:QT * (qi + 1)],
                    in_=ob[:])

        if os.environ.get("KERNEL_SEQ"):
            # baseline-like phase order (debug): all QKV, then attention+OP
            for sb in range(NSB):
                emit_qkv(sb, None)
            for qi in range(N_QT):
                otq = [otq_pool.tile([128, QT], f16, tag=f"otq{c}",
                                     name=f"otq{c}") for c in range(NK3)]
                for hp in range(NK3):
                    emit_pv_norm(qi, hp, emit_scores_exp(qi, hp), otq)
                emit_op(qi, otq)
        else:
            # one-stage software pipeline: PV/norm/OP for block sb are
            # emitted after block sb+1's QK+scores, so the next scores
            # (which feed the ACT engine) outrank them in the ready heap.
            pending = None
            for sb in range(NSB):
                otq = [otq_pool.tile([128, QT], f16, tag=f"otq{c}",
                                     name=f"otq{c}") for c in range(NK3)]
                pts_sb = [None] * NK3
                emit_qkv(sb, pts_sb)
                if pending is not None:
                    p_sb, p_pts, p_otq = pending
                    for hp in range(NK3):
                        emit_pv_norm(p_sb, hp, p_pts[hp], p_otq)
                    emit_op(p_sb, p_otq)
                pending = (sb, pts_sb, otq)
            p_sb, p_pts, p_otq = pending
            for hp in range(NK3):
                emit_pv_norm(p_sb, hp, p_pts[hp], p_otq)
            emit_op(p_sb, p_otq)
        if debug:
            for hp in range(NK3):
                for sb in range(NSB):
                    nc.sync.dma_start(out=dbg_q[hp, :, SB * sb:SB * (sb + 1)],
                                      in_=qkt_q[hp][sb][:])
                    nc.sync.dma_start(out=dbg_k[hp, :, SB * sb:SB * (sb + 1)],
                                      in_=qkt_k[hp][sb][:])
            for k in range(ND):
                nc.sync.dma_start(out=dbg_x[k], in_=xs[k][:])
            for t in range(N_KT):
                nc.sync.dma_start(out=dbg_v[t], in_=v_sb[t][:])
    nc.compile()
    return nc


def kernel(encodings_for_qkv, mask, w_qkv, b_qkv, w_o):
    global last_results
    from concourse.bass_utils import run_bass_kernel_spmd

    x = np.ascontiguousarray(np.asarray(encodings_for_qkv, dtype=np.float32))
    mask2d = np.asarray(mask).reshape(S, S).astype(bool)
    w_qkv = np.asarray(w_qkv, dtype=np.float32)
    b_qkv = np.asarray(b_qkv, dtype=np.float32)
    w_o = np.asarray(w_o, dtype=np.float32)

    plan, strips, biases = _classify_mask(mask2d)
    key = repr([[e[:3] + ((e[3][0], e[3][1]) if e[3] else None,) for e in row]
                for row in plan]) + repr(sorted(
                    (k, v) for k, v in os.environ.items() if k.startswith("KERNEL_")))
    if key not in _cache:
        _cache[key] = _build(plan, len(strips), len(biases))
    nc = _cache[key]

    maskm = (np.stack(strips) if strips
             else np.zeros((1, KTL, STRW), dtype=np.float32))
    maskb = (np.stack(biases) if biases
             else np.zeros((1, KTL, QT), dtype=np.float32))
    wT = np.ascontiguousarray(w_qkv.T)        # [D, 3D]
    woT_full = w_o.T                          # [D(in), D(out)]

    in_maps = []
    for c in range(8):
        b, g = divmod(c, 2)
        cols = slice(GD * g, GD * (g + 1))
        w_qk_g = np.ascontiguousarray(
            np.concatenate([wT[:, 0 * D:][:, cols], wT[:, 1 * D:][:, cols]], axis=1))
        b_q_g = np.ascontiguousarray(b_qkv[0 * D:1 * D][cols])
        w_v_g = np.ascontiguousarray(wT[:, 2 * D:][:, cols])
        wo_T_g = np.ascontiguousarray(woT_full[cols, :])
        in_maps.append({
            "xT": np.ascontiguousarray(x[b].T).astype(np.float16),
            "w_qk": w_qk_g.astype(np.float16), "b_q": b_q_g,
            "w_v": w_v_g.astype(np.float16),
            "wo_T": wo_T_g.astype(np.float16),
            "maskm": maskm.astype(np.float16), "maskb": maskb,
        })

    trace = bool(int(os.environ.get("KERNEL_PROFILE", "0")))
    res = run_bass_kernel_spmd(nc, in_maps, core_ids=list(range(8)),
                               trace=trace,
                               trace_cores=list(range(8)) if trace else None)
    last_results = res

    out = np.empty((B, S, D), dtype=np.float32)
    for b in range(B):
        acc = (res.results[2 * b]["outT"].astype(np.float32)
               + res.results[2 * b + 1]["outT"].astype(np.float32))
        out[b] = acc.T
    # V-bias epilogue: softmax rows sum to 1, so the V bias contributes a
    # constant (b_v @ w_o.T) to every sequence position.
    out += (b_qkv[2 * D:] @ woT_full).reshape(1, 1, D)
    return out
